# revision 9
# baseline (speedup 1.0000x reference)
"""Trainium2 Bass kernel for DictionaryModule (retrieval_knn).

Reference computation (per query row x of q_feats):
    h      = relu(x @ W1 + b1)
    qp     = h @ W2 + b2
    q      = qp / ||qp||
    k      = keys / ||keys|| (row-wise)
    sim    = q @ k.T                      [N, D]   (output 2)
    top5   = top_k(sim, 5)
    w      = softmax(top5_vals / 0.15)
    retr   = sum_t w_t * values[top5_idx_t]        (output 1)

Sharding: pure data-parallel over 8 NeuronCores; queries split N/8 per core,
dictionary + MLP weights replicated on every core.  No collectives.

Per-core plan:
  - MLP in fp32 on the PE with activations feature-major (x^T streamed in
    pre-transposed from the host), so W1/W2 native [in, out] are directly
    the stationary lhsT and no activation transposes are needed.
  - Query norms via square + ones-vector matmul (partition reduction on PE),
    applied as per-partition 1/||q|| scales at PSUM eviction.
  - Keys normalized dict-major (ACT square+accum), stored fp32 to DRAM for
    the rescue pass, and PE-transposed into a resident feature-major bf16
    copy for the sim matmul.
  - sim = q^T.T @ knT in bf16 (fp32 accumulate).  bf16 sim error (~2e-4) is
    >> fp32 top-k decision gaps, so the top-6 bf16 candidates (via the DVE
    max/max_index top-8 instructions) are re-scored exactly in fp32
    (indirect-DMA gather of normalized key rows + DVE dots), then top-5-of-6
    selected by threshold, softmaxed, and applied to gathered value rows.
"""

import sys

sys.path.insert(0, "/opt/trn_rl_repo")

from contextlib import ExitStack

import numpy as np

import concourse.bass as bass
import concourse.tile as tile
from concourse import bacc, mybir
from concourse.bass import IndirectOffsetOnAxis

F32 = mybir.dt.float32
BF16 = mybir.dt.bfloat16
U32 = mybir.dt.uint32
AF = mybir.ActivationFunctionType
ALU = mybir.AluOpType
AX = mybir.AxisListType

TEMPERATURE = 0.15
NEG_BIG = -1.0e30


class Cfg:
    def __init__(self, nq=2048, feat=1024, dict_size=8192, val=1024, topk=5,
                 n_cand=8):
        assert nq % 256 == 0 and feat % 128 == 0
        assert dict_size % 512 == 0 and val % 128 == 0
        self.NQ = nq              # queries per core
        self.F = feat             # feature/key dim
        self.D = dict_size        # dictionary entries
        self.V = val              # value dim
        self.TOPK = topk
        self.NC = n_cand          # rescued candidates (topk < n_cand <= 8)
        assert topk < n_cand <= 8 and n_cand % 2 == 0
        self.MT = nq // 128       # query tiles
        self.KC = feat // 128     # feature chunks (contraction)
        self.DB = dict_size // 512    # dict blocks of 512
        self.DT = dict_size // 128    # dict tiles of 128 rows
        self.HQ = nq // 2         # MLP query half-width
        self.SL = min(512, self.HQ)   # MLP moving-dim slice
        assert self.HQ % self.SL == 0
        self.NSH = self.HQ // self.SL  # slices per half
        self.MTH = self.HQ // 128      # query tiles per half


def build(cfg: Cfg):
    """Builds the single-core SPMD program. Returns the bacc module."""
    nc = bacc.Bacc("TRN2", target_bir_lowering=False, debug=False)
    c = cfg

    io = {}
    io["xT"] = nc.dram_tensor("xT", [c.F, c.NQ], F32, kind="ExternalInput").ap()
    io["w1"] = nc.dram_tensor("w1", [c.F, c.F], F32, kind="ExternalInput").ap()
    io["w2"] = nc.dram_tensor("w2", [c.F, c.F], F32, kind="ExternalInput").ap()
    io["b1"] = nc.dram_tensor("b1", [128, c.KC], F32, kind="ExternalInput").ap()
    io["b2"] = nc.dram_tensor("b2", [128, c.KC], F32, kind="ExternalInput").ap()
    io["keys"] = nc.dram_tensor("keys", [c.D, c.F], F32,
                                kind="ExternalInput").ap()
    io["values"] = nc.dram_tensor("values", [c.D, c.V], F32,
                                  kind="ExternalInput").ap()
    io["ident"] = nc.dram_tensor("ident", [128, 128], F32,
                                 kind="ExternalInput").ap()
    io["sim_out"] = nc.dram_tensor("sim_out", [c.NQ, c.D], F32,
                                   kind="ExternalOutput").ap()
    io["retr_out"] = nc.dram_tensor("retr_out", [c.NQ, c.V], F32,
                                    kind="ExternalOutput").ap()
    # DRAM scratch
    io["kn_dict"] = nc.dram_tensor("kn_dict", [c.D, c.F], F32).ap()
    io["qt_b16"] = nc.dram_tensor("qt_b16", [c.F, c.NQ], BF16).ap()
    io["qqm_dram"] = nc.dram_tensor("qqm_dram", [c.NQ, c.F], F32).ap()
    io["ssq_dram"] = nc.dram_tensor("ssq_dram", [c.MT, 128], F32).ap()

    with tile.TileContext(nc) as tc:
        _emit(tc, c, io)
    nc.compile()
    return nc


def _emit(tc, c, io):
    nc = tc.nc
    xT, w1, w2, b1, b2 = io["xT"], io["w1"], io["w2"], io["b1"], io["b2"]
    keys, values, ident = io["keys"], io["values"], io["ident"]
    sim_out, retr_out = io["sim_out"], io["retr_out"]
    kn_dict, qt_b16 = io["kn_dict"], io["qt_b16"]
    qqm_dram, ssq_dram = io["qqm_dram"], io["ssq_dram"]

    with ExitStack() as top:
        const_pool = top.enter_context(tc.tile_pool(name="const", bufs=1))
        rq_pool = top.enter_context(tc.tile_pool(name="rq", bufs=1))

        ident_t = const_pool.tile([128, 128], F32, tag="ident")
        nc.sync.dma_start(out=ident_t[:], in_=ident[:, :])
        ones_t = const_pool.tile([128, 1], F32, tag="ones")
        nc.vector.memset(ones_t[:], 1.0)

        # per-query 1/||q||, column layout: rq_col[p, m] = 1/||q_{m*128+p}||
        rq_col = rq_pool.tile([128, c.MT], F32, tag="rq_col")

        # ================= Phase 1: MLP (fp32, feature-major) =============
        with ExitStack() as ph:
            x_pool = ph.enter_context(tc.tile_pool(name="x", bufs=1))
            h_pool = ph.enter_context(tc.tile_pool(name="h", bufs=1))
            qp_pool = ph.enter_context(tc.tile_pool(name="qp", bufs=1))
            wgt_pool = ph.enter_context(tc.tile_pool(name="wgt", bufs=2))
            bias_pool = ph.enter_context(tc.tile_pool(name="bias", bufs=1))
            sq_pool = ph.enter_context(tc.tile_pool(name="sq", bufs=1))
            qtb_pool = ph.enter_context(tc.tile_pool(name="qtb", bufs=2))
            ssq_pool = ph.enter_context(tc.tile_pool(name="ssq", bufs=1))
            qqm_pool = ph.enter_context(tc.tile_pool(name="qqm", bufs=2))
            mlp_ps = ph.enter_context(
                tc.tile_pool(name="mlp_ps", bufs=2, space="PSUM"))
            ssq_ps = ph.enter_context(
                tc.tile_pool(name="ssq_ps", bufs=1, space="PSUM"))
            tr_ps = ph.enter_context(
                tc.tile_pool(name="tr_ps", bufs=2, space="PSUM"))

            b1_sb = bias_pool.tile([128, c.KC], F32, tag="b1")
            nc.sync.dma_start(out=b1_sb[:], in_=b1[:, :])
            b2_sb = bias_pool.tile([128, c.KC], F32, tag="b2")
            nc.sync.dma_start(out=b2_sb[:], in_=b2[:, :])

            for hq in range(2):
                qcols = slice(hq * c.HQ, (hq + 1) * c.HQ)
                x_sb = x_pool.tile([128, c.KC, c.HQ], F32, tag="x")
                nc.sync.dma_start(
                    out=x_sb[:],
                    in_=xT.rearrange("(k p) n -> p k n", p=128)[:, :, qcols])

                # ---- layer 1 ----
                h_sb = h_pool.tile([128, c.KC, c.HQ], F32, tag="h")
                for o in range(c.KC):
                    wt = wgt_pool.tile([128, c.KC, 128], F32, tag="w")
                    nc.sync.dma_start(
                        out=wt[:],
                        in_=w1.rearrange("(k p) o -> p k o", p=128)
                        [:, :, o * 128:(o + 1) * 128])
                    for n in range(c.NSH):
                        ns = slice(n * c.SL, (n + 1) * c.SL)
                        ps = mlp_ps.tile([128, c.SL], F32, tag="mlp_ps")
                        for k in range(c.KC):
                            nc.tensor.matmul(
                                out=ps[:], lhsT=wt[:, k, :],
                                rhs=x_sb[:, k, ns],
                                start=(k == 0), stop=(k == c.KC - 1))
                        nc.scalar.activation(
                            out=h_sb[:, o, ns], in_=ps[:],
                            func=AF.Relu, bias=b1_sb[:, o:o + 1], scale=1.0)

                # ---- layer 2 (+ query squared-norms via ones-matmul) ----
                qp_sb = qp_pool.tile([128, c.KC, c.HQ], F32, tag="qp")
                sq_sb = sq_pool.tile([128, c.HQ], F32, tag="sq")
                ssq_psum = [ssq_ps.tile([1, c.SL], F32, tag=f"ssq{n}",
                                        name=f"ssq_psum{n}")
                            for n in range(c.NSH)]
                for o in range(c.KC):
                    wt = wgt_pool.tile([128, c.KC, 128], F32, tag="w")
                    nc.sync.dma_start(
                        out=wt[:],
                        in_=w2.rearrange("(k p) o -> p k o", p=128)
                        [:, :, o * 128:(o + 1) * 128])
                    for n in range(c.NSH):
                        ns = slice(n * c.SL, (n + 1) * c.SL)
                        ps = mlp_ps.tile([128, c.SL], F32, tag="mlp_ps")
                        for k in range(c.KC):
                            nc.tensor.matmul(
                                out=ps[:], lhsT=wt[:, k, :],
                                rhs=h_sb[:, k, ns],
                                start=(k == 0), stop=(k == c.KC - 1))
                        nc.scalar.activation(
                            out=qp_sb[:, o, ns], in_=ps[:],
                            func=AF.Identity, bias=b2_sb[:, o:o + 1],
                            scale=1.0)
                    nc.vector.tensor_tensor(
                        out=sq_sb[:], in0=qp_sb[:, o, :], in1=qp_sb[:, o, :],
                        op=ALU.mult)
                    for n in range(c.NSH):
                        ns = slice(n * c.SL, (n + 1) * c.SL)
                        nc.tensor.matmul(
                            out=ssq_psum[n][:], lhsT=ones_t[:],
                            rhs=sq_sb[:, ns],
                            start=(o == 0), stop=(o == c.KC - 1))

                # ssq -> rq_col slice for this half (via DRAM re-layout)
                ssq_row = ssq_pool.tile([1, c.HQ], F32, tag="ssq_row")
                for n in range(c.NSH):
                    nc.scalar.activation(
                        out=ssq_row[:, n * c.SL:(n + 1) * c.SL],
                        in_=ssq_psum[n][:], func=AF.Copy)
                nc.sync.dma_start(
                    out=ssq_dram[hq * c.MTH:(hq + 1) * c.MTH, :],
                    in_=ssq_row[:])
                ssq_col = ssq_pool.tile([128, c.MTH], F32, tag="ssq_col")
                nc.sync.dma_start(
                    out=ssq_col[:],
                    in_=ssq_dram[hq * c.MTH:(hq + 1) * c.MTH, :]
                    .rearrange("m p -> p m"))
                nrm_col = ssq_pool.tile([128, c.MTH], F32, tag="nrm_col")
                nc.scalar.activation(out=nrm_col[:], in_=ssq_col[:],
                                     func=AF.Sqrt)
                nc.vector.reciprocal(
                    out=rq_col[:, hq * c.MTH:(hq + 1) * c.MTH],
                    in_=nrm_col[:])

                # q^T -> bf16 -> DRAM (sim lhsT, streamed back per m-block)
                for k in range(c.KC):
                    qtb_sb = qtb_pool.tile([128, c.HQ], BF16, tag="qtb")
                    nc.vector.tensor_copy(out=qtb_sb[:], in_=qp_sb[:, k, :])
                    nc.sync.dma_start(
                        out=qt_b16[k * 128:(k + 1) * 128, qcols],
                        in_=qtb_sb[:])

                # query-major normalized q -> DRAM (fp32 rescue operand)
                for mh in range(c.MTH):
                    m = hq * c.MTH + mh
                    qqm_t = qqm_pool.tile([128, c.F], F32, tag="qqm")
                    for f in range(c.KC):
                        pt = tr_ps.tile([128, 128], F32, tag="tr")
                        nc.tensor.transpose(
                            out=pt[:],
                            in_=qp_sb[:, f, mh * 128:(mh + 1) * 128],
                            identity=ident_t[:])
                        nc.scalar.activation(
                            out=qqm_t[:, f * 128:(f + 1) * 128], in_=pt[:],
                            func=AF.Copy, scale=rq_col[:, m:m + 1])
                    nc.sync.dma_start(
                        out=qqm_dram[m * 128:(m + 1) * 128, :], in_=qqm_t[:])

        # ===================== Phases 2+3 (knT resident) =====================
        knT_pool = top.enter_context(tc.tile_pool(name="knT", bufs=1))
        # resident feature-major normalized keys (bf16): [128, KC, D]
        knT = knT_pool.tile([128, c.KC, c.D], BF16, tag="knT")

        # ===================== Phase 2: key prep =====================
        with ExitStack() as ph:
            kt_pool = ph.enter_context(tc.tile_pool(name="kt", bufs=3))
            kn_pool = ph.enter_context(tc.tile_pool(name="kn", bufs=3))
            ksc_pool = ph.enter_context(tc.tile_pool(name="ksc", bufs=3))
            ktr_ps = ph.enter_context(
                tc.tile_pool(name="ktr_ps", bufs=4, space="PSUM"))
            for j in range(c.DT):
                kt = kt_pool.tile([128, c.F], F32, tag="kt")
                nc.sync.dma_start(
                    out=kt[:], in_=keys[j * 128:(j + 1) * 128, :])
                ksq = ksc_pool.tile([128, c.F], F32, tag="ksq")
                kssq = ksc_pool.tile([128, 1], F32, tag="kssq")
                nc.scalar.activation(out=ksq[:], in_=kt[:], func=AF.Square,
                                     accum_out=kssq[:])
                knorm = ksc_pool.tile([128, 1], F32, tag="knorm")
                nc.scalar.activation(out=knorm[:], in_=kssq[:], func=AF.Sqrt)
                krk = ksc_pool.tile([128, 1], F32, tag="krk")
                nc.vector.reciprocal(out=krk[:], in_=knorm[:])
                kn = kn_pool.tile([128, c.F], F32, tag="kn")
                nc.scalar.activation(out=kn[:], in_=kt[:], func=AF.Copy,
                                     scale=krk[:, 0:1])
                nc.sync.dma_start(
                    out=kn_dict[j * 128:(j + 1) * 128, :], in_=kn[:])
                for f in range(c.KC):
                    pt = ktr_ps.tile([128, 128], F32, tag="ktr")
                    nc.tensor.transpose(
                        out=pt[:], in_=kn[:, f * 128:(f + 1) * 128],
                        identity=ident_t[:])
                    nc.any.tensor_copy(
                        out=knT[:, f, j * 128:(j + 1) * 128], in_=pt[:])

        # ========= Phase 3: sim + topk + rescue + retrieve =========
        with ExitStack() as ph:
            qt_pool = ph.enter_context(tc.tile_pool(name="qt", bufs=2))
            qq_pool = ph.enter_context(tc.tile_pool(name="qq", bufs=2))
            sim_pool = ph.enter_context(tc.tile_pool(name="simt", bufs=1))
            kv_pool = ph.enter_context(tc.tile_pool(name="kv", bufs=1))
            sm_pool = ph.enter_context(tc.tile_pool(name="sm", bufs=2))
            ret_pool = ph.enter_context(tc.tile_pool(name="ret", bufs=1))
            ttr_pool = ph.enter_context(tc.tile_pool(name="ttr", bufs=1))
            sim_ps = ph.enter_context(
                tc.tile_pool(name="sim_ps", bufs=4, space="PSUM"))

            NCAND = c.NC
            half = NCAND // 2
            for m in range(c.MT):
                qtb = qt_pool.tile([128, c.KC, 128], BF16, tag="qtb")
                nc.sync.dma_start(
                    out=qtb[:],
                    in_=qt_b16.rearrange("(k p) n -> p k n", p=128)
                    [:, :, m * 128:(m + 1) * 128])
                qqm_t = qq_pool.tile([128, c.F], F32, tag="qqm")
                nc.sync.dma_start(
                    out=qqm_t[:], in_=qqm_dram[m * 128:(m + 1) * 128, :])

                sim_t = sim_pool.tile([128, c.D], F32, tag="sim")
                for d in range(c.DB):
                    ds = slice(d * 512, (d + 1) * 512)
                    ps = sim_ps.tile([128, 512], F32, tag="sim_ps")
                    for k in range(c.KC):
                        nc.tensor.matmul(
                            out=ps[:], lhsT=qtb[:, k, :],
                            rhs=knT[:, k, ds],
                            start=(k == 0), stop=(k == c.KC - 1))
                    nc.scalar.activation(
                        out=sim_t[:, ds], in_=ps[:],
                        func=AF.Copy, scale=rq_col[:, m:m + 1])
                    nc.sync.dma_start(
                        out=sim_out[m * 128:(m + 1) * 128, ds],
                        in_=sim_t[:, ds])

                # top-8 (descending) values + indices per row
                vals8 = sm_pool.tile([128, 8], F32, tag="vals8")
                nc.vector.max(out=vals8[:], in_=sim_t[:])
                idx8 = sm_pool.tile([128, 8], U32, tag="idx8")
                nc.vector.max_index(out=idx8[:], in_max=vals8[:],
                                    in_values=sim_t[:])

                # rescue: gather top-NCAND normalized key rows, fp32 dots
                rdots = sm_pool.tile([128, 8], F32, tag="rdots")
                nc.vector.memset(rdots[:], NEG_BIG)
                ttr_o = ttr_pool.tile([128, c.F], F32, tag="ttr_o")
                for hf in range(2):
                    kvb = kv_pool.tile([128, half, c.F], F32, tag="kvb")
                    for t in range(half):
                        tt = hf * half + t
                        nc.gpsimd.indirect_dma_start(
                            out=kvb[:, t, :], out_offset=None,
                            in_=kn_dict,
                            in_offset=IndirectOffsetOnAxis(
                                ap=idx8[:, tt:tt + 1], axis=0))
                    for t in range(half):
                        tt = hf * half + t
                        nc.vector.tensor_tensor(
                            out=ttr_o[:], in0=kvb[:, t, :], in1=qqm_t[:],
                            op=ALU.mult)
                        nc.vector.tensor_reduce(
                            out=rdots[:, tt:tt + 1], in_=ttr_o[:],
                            axis=AX.X, op=ALU.add)

                # top-5-of-NCAND selection + softmax weights
                sort8 = sm_pool.tile([128, 8], F32, tag="sort8")
                nc.vector.max(out=sort8[:], in_=rdots[:])
                ebias = sm_pool.tile([128, 1], F32, tag="ebias")
                nc.vector.tensor_scalar_mul(
                    ebias[:], sort8[:, 0:1], -1.0 / TEMPERATURE)
                mask = sm_pool.tile([128, NCAND], F32, tag="mask")
                nc.vector.tensor_scalar(
                    out=mask[:], in0=rdots[:, 0:NCAND],
                    scalar1=sort8[:, c.TOPK - 1:c.TOPK], scalar2=None,
                    op0=ALU.is_ge)
                ew = sm_pool.tile([128, NCAND], F32, tag="ew")
                nc.scalar.activation(
                    out=ew[:], in_=rdots[:, 0:NCAND], func=AF.Exp,
                    bias=ebias[:, 0:1], scale=1.0 / TEMPERATURE)
                ewm = sm_pool.tile([128, NCAND], F32, tag="ewm")
                nc.vector.tensor_tensor(out=ewm[:], in0=ew[:], in1=mask[:],
                                        op=ALU.mult)
                den = sm_pool.tile([128, 1], F32, tag="den")
                nc.vector.tensor_reduce(out=den[:], in_=ewm[:], axis=AX.X,
                                        op=ALU.add)
                rden = sm_pool.tile([128, 1], F32, tag="rden")
                nc.vector.reciprocal(out=rden[:], in_=den[:])
                wgt = sm_pool.tile([128, NCAND], F32, tag="wgt")
                nc.vector.tensor_scalar_mul(wgt[:], ewm[:], rden[:, 0:1])

                # gather value rows, scale by weights, accumulate
                ret_t = ret_pool.tile([128, c.V], F32, tag="ret_t")
                for hf in range(2):
                    kvb = kv_pool.tile([128, half, c.V], F32, tag="kvb")
                    for t in range(half):
                        tt = hf * half + t
                        nc.gpsimd.indirect_dma_start(
                            out=kvb[:, t, :], out_offset=None,
                            in_=values,
                            in_offset=IndirectOffsetOnAxis(
                                ap=idx8[:, tt:tt + 1], axis=0))
                    for t in range(half):
                        tt = hf * half + t
                        nc.scalar.activation(
                            out=kvb[:, t, :], in_=kvb[:, t, :], func=AF.Copy,
                            scale=wgt[:, tt:tt + 1])
                    for t in range(half):
                        if hf == 0 and t == 0:
                            continue
                        elif hf == 0 and t == 1:
                            nc.vector.tensor_tensor(
                                out=ret_t[:], in0=kvb[:, 0, :],
                                in1=kvb[:, 1, :], op=ALU.add)
                        else:
                            nc.vector.tensor_tensor(
                                out=ret_t[:], in0=ret_t[:],
                                in1=kvb[:, t, :], op=ALU.add)
                nc.sync.dma_start(
                    out=retr_out[m * 128:(m + 1) * 128, :], in_=ret_t[:])


# ======================= host-side entry =======================

_CACHED = {}


def make_in_maps(cfg, q_feats, W1, b1, W2, b2, keys_p, values_p, n_cores=8):
    shard = q_feats.shape[0] // n_cores
    b1_t = np.ascontiguousarray(
        np.asarray(b1, np.float32).reshape(cfg.KC, 128).T)
    b2_t = np.ascontiguousarray(
        np.asarray(b2, np.float32).reshape(cfg.KC, 128).T)
    ident = np.eye(128, dtype=np.float32)
    w1 = np.ascontiguousarray(W1, np.float32)
    w2 = np.ascontiguousarray(W2, np.float32)
    keys = np.ascontiguousarray(keys_p, np.float32)
    vals = np.ascontiguousarray(values_p, np.float32)
    maps = []
    for i in range(n_cores):
        xs = np.asarray(q_feats[i * shard:(i + 1) * shard], np.float32)
        maps.append({
            "xT": np.ascontiguousarray(xs.T),
            "w1": w1, "w2": w2, "b1": b1_t, "b2": b2_t,
            "keys": keys, "values": vals, "ident": ident,
        })
    return maps


def kernel(q_feats, W1, b1, W2, b2, keys_p, values_p, topk):
    from concourse.bass_utils import run_bass_kernel_spmd

    assert int(topk) == 5, f"kernel hardcodes topk=5, got {topk}"
    N_CORES = 8
    N, F = q_feats.shape
    D = keys_p.shape[0]
    V = values_p.shape[1]
    assert (N, F, D, V) == (16384, 1024, 8192, 1024)

    cfg = Cfg()
    if "nc" not in _CACHED:
        _CACHED["nc"] = build(cfg)
    nc = _CACHED["nc"]

    in_maps = make_in_maps(cfg, q_feats, W1, b1, W2, b2, keys_p, values_p,
                           N_CORES)
    res = run_bass_kernel_spmd(nc, in_maps, core_ids=list(range(N_CORES)))
    sim = np.concatenate(
        [res.results[i]["sim_out"] for i in range(N_CORES)], axis=0)
    retr = np.concatenate(
        [res.results[i]["retr_out"] for i in range(N_CORES)], axis=0)
    return retr.astype(np.float32, copy=False), sim.astype(np.float32,
                                                           copy=False)


# revision 13
# speedup vs baseline: 1.0285x; 1.0285x over previous
"""Trainium2 Bass kernel for DictionaryModule (retrieval_knn).

Reference computation (per query row x of q_feats):
    h      = relu(x @ W1 + b1)
    qp     = h @ W2 + b2
    q      = qp / ||qp||
    k      = keys / ||keys|| (row-wise)
    sim    = q @ k.T                      [N, D]   (output 2)
    top5   = top_k(sim, 5)
    w      = softmax(top5_vals / 0.15)
    retr   = sum_t w_t * values[top5_idx_t]        (output 1)

Sharding: pure data-parallel over 8 NeuronCores; queries split N/8 per core,
dictionary + MLP weights replicated on every core.  No collectives.

Per-core plan:
  - MLP in fp32 on the PE with activations feature-major (x^T streamed in
    pre-transposed from the host), so W1/W2 native [in, out] are directly
    the stationary lhsT and no activation transposes are needed.
  - Query norms via square + ones-vector matmul (partition reduction on PE),
    applied as per-partition 1/||q|| scales at PSUM eviction.
  - Keys normalized dict-major (ACT square+accum), stored fp32 to DRAM for
    the rescue pass, and PE-transposed into a resident feature-major bf16
    copy for the sim matmul.
  - sim = q^T.T @ knT in bf16 (fp32 accumulate).  bf16 sim error (~2e-4) is
    >> fp32 top-k decision gaps, so the top-6 bf16 candidates (via the DVE
    max/max_index top-8 instructions) are re-scored exactly in fp32
    (indirect-DMA gather of normalized key rows + DVE dots), then top-5-of-6
    selected by threshold, softmaxed, and applied to gathered value rows.
"""

import sys

sys.path.insert(0, "/opt/trn_rl_repo")

from contextlib import ExitStack

import numpy as np

import concourse.bass as bass
import concourse.tile as tile
from concourse import bacc, mybir
from concourse.bass import IndirectOffsetOnAxis

F32 = mybir.dt.float32
BF16 = mybir.dt.bfloat16
U32 = mybir.dt.uint32
AF = mybir.ActivationFunctionType
ALU = mybir.AluOpType
AX = mybir.AxisListType

TEMPERATURE = 0.15
NEG_BIG = -1.0e30


class Cfg:
    def __init__(self, nq=2048, feat=1024, dict_size=8192, val=1024, topk=5,
                 n_cand=8):
        assert nq % 256 == 0 and feat % 128 == 0
        assert dict_size % 512 == 0 and val % 128 == 0
        self.NQ = nq              # queries per core
        self.F = feat             # feature/key dim
        self.D = dict_size        # dictionary entries
        self.V = val              # value dim
        self.TOPK = topk
        self.NC = n_cand          # rescued candidates (topk < n_cand <= 8)
        assert topk < n_cand <= 8 and n_cand % 2 == 0
        self.MT = nq // 128       # query tiles
        self.KC = feat // 128     # feature chunks (contraction)
        self.DB = dict_size // 512    # dict blocks of 512
        self.DT = dict_size // 128    # dict tiles of 128 rows
        self.HQ = nq // 2         # MLP query half-width
        self.SL = min(512, self.HQ)   # MLP moving-dim slice
        assert self.HQ % self.SL == 0
        self.NSH = self.HQ // self.SL  # slices per half
        self.MTH = self.HQ // 128      # query tiles per half


def build(cfg: Cfg):
    """Builds the single-core SPMD program. Returns the bacc module."""
    nc = bacc.Bacc("TRN2", target_bir_lowering=False, debug=False)
    c = cfg

    io = {}
    io["xT"] = nc.dram_tensor("xT", [c.F, c.NQ], F32, kind="ExternalInput").ap()
    io["w1"] = nc.dram_tensor("w1", [c.F, c.F], F32, kind="ExternalInput").ap()
    io["w2"] = nc.dram_tensor("w2", [c.F, c.F], F32, kind="ExternalInput").ap()
    io["b1"] = nc.dram_tensor("b1", [128, c.KC], F32, kind="ExternalInput").ap()
    io["b2"] = nc.dram_tensor("b2", [128, c.KC], F32, kind="ExternalInput").ap()
    io["keys"] = nc.dram_tensor("keys", [c.D, c.F], F32,
                                kind="ExternalInput").ap()
    io["values"] = nc.dram_tensor("values", [c.D, c.V], F32,
                                  kind="ExternalInput").ap()
    io["ident"] = nc.dram_tensor("ident", [128, 128], F32,
                                 kind="ExternalInput").ap()
    io["sim_out"] = nc.dram_tensor("sim_out", [c.NQ, c.D], F32,
                                   kind="ExternalOutput").ap()
    io["retr_out"] = nc.dram_tensor("retr_out", [c.NQ, c.V], F32,
                                    kind="ExternalOutput").ap()
    # DRAM scratch
    io["kn_dict"] = nc.dram_tensor("kn_dict", [c.D, c.F], F32).ap()
    io["knb_dram"] = nc.dram_tensor("knb_dram", [c.D, c.F], BF16).ap()
    io["vb_dram"] = nc.dram_tensor("vb_dram", [c.D, c.V], BF16).ap()
    io["qt_b16"] = nc.dram_tensor("qt_b16", [c.F, c.NQ], BF16).ap()
    io["qqm_dram"] = nc.dram_tensor("qqm_dram", [c.NQ, c.F], F32).ap()
    io["ssq_dram"] = nc.dram_tensor("ssq_dram", [c.MT, 128], F32).ap()

    with tile.TileContext(nc) as tc:
        _emit(tc, c, io)
    nc.compile()
    return nc


def _emit(tc, c, io):
    nc = tc.nc
    xT, w1, w2, b1, b2 = io["xT"], io["w1"], io["w2"], io["b1"], io["b2"]
    keys, values, ident = io["keys"], io["values"], io["ident"]
    sim_out, retr_out = io["sim_out"], io["retr_out"]
    kn_dict, qt_b16 = io["kn_dict"], io["qt_b16"]
    qqm_dram, ssq_dram = io["qqm_dram"], io["ssq_dram"]
    knb_dram, vb_dram = io["knb_dram"], io["vb_dram"]

    with ExitStack() as top:
        const_pool = top.enter_context(tc.tile_pool(name="const", bufs=1))
        rq_pool = top.enter_context(tc.tile_pool(name="rq", bufs=1))

        ident_t = const_pool.tile([128, 128], F32, tag="ident")
        nc.sync.dma_start(out=ident_t[:], in_=ident[:, :])
        ones_t = const_pool.tile([128, 1], F32, tag="ones")
        nc.vector.memset(ones_t[:], 1.0)

        # per-query 1/||q||, column layout: rq_col[p, m] = 1/||q_{m*128+p}||
        rq_col = rq_pool.tile([128, c.MT], F32, tag="rq_col")

        # ================= Phase 1: MLP (fp32, feature-major) =============
        with ExitStack() as ph:
            x_pool = ph.enter_context(tc.tile_pool(name="x", bufs=1))
            h_pool = ph.enter_context(tc.tile_pool(name="h", bufs=1))
            qp_pool = ph.enter_context(tc.tile_pool(name="qp", bufs=1))
            wgt_pool = ph.enter_context(tc.tile_pool(name="wgt", bufs=2))
            bias_pool = ph.enter_context(tc.tile_pool(name="bias", bufs=1))
            sq_pool = ph.enter_context(tc.tile_pool(name="sq", bufs=1))
            qtb_pool = ph.enter_context(tc.tile_pool(name="qtb", bufs=2))
            ssq_pool = ph.enter_context(tc.tile_pool(name="ssq", bufs=1))
            qqm_pool = ph.enter_context(tc.tile_pool(name="qqm", bufs=2))
            mlp_ps = ph.enter_context(
                tc.tile_pool(name="mlp_ps", bufs=2, space="PSUM"))
            ssq_ps = ph.enter_context(
                tc.tile_pool(name="ssq_ps", bufs=1, space="PSUM"))
            tr_ps = ph.enter_context(
                tc.tile_pool(name="tr_ps", bufs=2, space="PSUM"))

            b1_sb = bias_pool.tile([128, c.KC], F32, tag="b1")
            nc.sync.dma_start(out=b1_sb[:], in_=b1[:, :])
            b2_sb = bias_pool.tile([128, c.KC], F32, tag="b2")
            nc.sync.dma_start(out=b2_sb[:], in_=b2[:, :])

            for hq in range(2):
                qcols = slice(hq * c.HQ, (hq + 1) * c.HQ)
                x_sb = x_pool.tile([128, c.KC, c.HQ], F32, tag="x")
                nc.sync.dma_start(
                    out=x_sb[:],
                    in_=xT.rearrange("(k p) n -> p k n", p=128)[:, :, qcols])

                # ---- layer 1 ----
                h_sb = h_pool.tile([128, c.KC, c.HQ], F32, tag="h")
                for o in range(c.KC):
                    wt = wgt_pool.tile([128, c.KC, 128], F32, tag="w")
                    nc.sync.dma_start(
                        out=wt[:],
                        in_=w1.rearrange("(k p) o -> p k o", p=128)
                        [:, :, o * 128:(o + 1) * 128])
                    for n in range(c.NSH):
                        ns = slice(n * c.SL, (n + 1) * c.SL)
                        ps = mlp_ps.tile([128, c.SL], F32, tag="mlp_ps")
                        for k in range(c.KC):
                            nc.tensor.matmul(
                                out=ps[:], lhsT=wt[:, k, :],
                                rhs=x_sb[:, k, ns],
                                start=(k == 0), stop=(k == c.KC - 1))
                        nc.scalar.activation(
                            out=h_sb[:, o, ns], in_=ps[:],
                            func=AF.Relu, bias=b1_sb[:, o:o + 1], scale=1.0)

                # ---- layer 2 (+ query squared-norms via ones-matmul) ----
                qp_sb = qp_pool.tile([128, c.KC, c.HQ], F32, tag="qp")
                sq_sb = sq_pool.tile([128, c.HQ], F32, tag="sq")
                ssq_psum = [ssq_ps.tile([1, c.SL], F32, tag=f"ssq{n}",
                                        name=f"ssq_psum{n}")
                            for n in range(c.NSH)]
                for o in range(c.KC):
                    wt = wgt_pool.tile([128, c.KC, 128], F32, tag="w")
                    nc.sync.dma_start(
                        out=wt[:],
                        in_=w2.rearrange("(k p) o -> p k o", p=128)
                        [:, :, o * 128:(o + 1) * 128])
                    for n in range(c.NSH):
                        ns = slice(n * c.SL, (n + 1) * c.SL)
                        ps = mlp_ps.tile([128, c.SL], F32, tag="mlp_ps")
                        for k in range(c.KC):
                            nc.tensor.matmul(
                                out=ps[:], lhsT=wt[:, k, :],
                                rhs=h_sb[:, k, ns],
                                start=(k == 0), stop=(k == c.KC - 1))
                        nc.scalar.activation(
                            out=qp_sb[:, o, ns], in_=ps[:],
                            func=AF.Identity, bias=b2_sb[:, o:o + 1],
                            scale=1.0)
                    nc.vector.tensor_tensor(
                        out=sq_sb[:], in0=qp_sb[:, o, :], in1=qp_sb[:, o, :],
                        op=ALU.mult)
                    for n in range(c.NSH):
                        ns = slice(n * c.SL, (n + 1) * c.SL)
                        nc.tensor.matmul(
                            out=ssq_psum[n][:], lhsT=ones_t[:],
                            rhs=sq_sb[:, ns],
                            start=(o == 0), stop=(o == c.KC - 1))

                # ssq -> rq_col slice for this half (via DRAM re-layout)
                ssq_row = ssq_pool.tile([1, c.HQ], F32, tag="ssq_row")
                for n in range(c.NSH):
                    nc.scalar.activation(
                        out=ssq_row[:, n * c.SL:(n + 1) * c.SL],
                        in_=ssq_psum[n][:], func=AF.Copy)
                nc.sync.dma_start(
                    out=ssq_dram[hq * c.MTH:(hq + 1) * c.MTH, :],
                    in_=ssq_row[:])
                ssq_col = ssq_pool.tile([128, c.MTH], F32, tag="ssq_col")
                nc.sync.dma_start(
                    out=ssq_col[:],
                    in_=ssq_dram[hq * c.MTH:(hq + 1) * c.MTH, :]
                    .rearrange("m p -> p m"))
                nrm_col = ssq_pool.tile([128, c.MTH], F32, tag="nrm_col")
                nc.scalar.activation(out=nrm_col[:], in_=ssq_col[:],
                                     func=AF.Sqrt)
                nc.vector.reciprocal(
                    out=rq_col[:, hq * c.MTH:(hq + 1) * c.MTH],
                    in_=nrm_col[:])

                # q^T -> bf16 -> DRAM (sim lhsT, streamed back per m-block)
                for k in range(c.KC):
                    qtb_sb = qtb_pool.tile([128, c.HQ], BF16, tag="qtb")
                    nc.vector.tensor_copy(out=qtb_sb[:], in_=qp_sb[:, k, :])
                    nc.sync.dma_start(
                        out=qt_b16[k * 128:(k + 1) * 128, qcols],
                        in_=qtb_sb[:])

                # query-major normalized q -> DRAM (fp32 rescue operand)
                for mh in range(c.MTH):
                    m = hq * c.MTH + mh
                    qqm_t = qqm_pool.tile([128, c.F], F32, tag="qqm")
                    for f in range(c.KC):
                        pt = tr_ps.tile([128, 128], F32, tag="tr")
                        nc.tensor.transpose(
                            out=pt[:],
                            in_=qp_sb[:, f, mh * 128:(mh + 1) * 128],
                            identity=ident_t[:])
                        nc.scalar.activation(
                            out=qqm_t[:, f * 128:(f + 1) * 128], in_=pt[:],
                            func=AF.Copy, scale=rq_col[:, m:m + 1])
                    nc.sync.dma_start(
                        out=qqm_dram[m * 128:(m + 1) * 128, :], in_=qqm_t[:])

        # ===================== Phases 2+3 (knT resident) =====================
        knT_pool = top.enter_context(tc.tile_pool(name="knT", bufs=1))
        # resident feature-major normalized keys (bf16): [128, KC, D]
        knT = knT_pool.tile([128, c.KC, c.D], BF16, tag="knT")

        # ======= Phase 2: key normalize + bf16 tables (no PE work) =======
        with ExitStack() as ph:
            kt_pool = ph.enter_context(tc.tile_pool(name="kt", bufs=3))
            kn_pool = ph.enter_context(tc.tile_pool(name="kn", bufs=3))
            ksc_pool = ph.enter_context(tc.tile_pool(name="ksc", bufs=3))
            vt_pool = ph.enter_context(tc.tile_pool(name="vt", bufs=3))
            for j in range(c.DT):
                kt = kt_pool.tile([128, c.F], F32, tag="kt")
                nc.sync.dma_start(
                    out=kt[:], in_=keys[j * 128:(j + 1) * 128, :])
                ksq = ksc_pool.tile([128, c.F], F32, tag="ksq")
                kssq = ksc_pool.tile([128, 1], F32, tag="kssq")
                nc.scalar.activation(out=ksq[:], in_=kt[:], func=AF.Square,
                                     accum_out=kssq[:])
                knorm = ksc_pool.tile([128, 1], F32, tag="knorm")
                nc.scalar.activation(out=knorm[:], in_=kssq[:], func=AF.Sqrt)
                krk = ksc_pool.tile([128, 1], F32, tag="krk")
                nc.vector.reciprocal(out=krk[:], in_=knorm[:])
                kn = kn_pool.tile([128, c.F], F32, tag="kn")
                nc.scalar.activation(out=kn[:], in_=kt[:], func=AF.Copy,
                                     scale=krk[:, 0:1])
                nc.sync.dma_start(
                    out=kn_dict[j * 128:(j + 1) * 128, :], in_=kn[:])
                knb = kn_pool.tile([128, c.F], BF16, tag="knb")
                nc.vector.tensor_copy(out=knb[:], in_=kn[:])
                nc.sync.dma_start(
                    out=knb_dram[j * 128:(j + 1) * 128, :], in_=knb[:])
                vt = vt_pool.tile([128, c.V], F32, tag="vt")
                nc.sync.dma_start(
                    out=vt[:], in_=values[j * 128:(j + 1) * 128, :])
                vb = vt_pool.tile([128, c.V], BF16, tag="vb")
                nc.vector.tensor_copy(out=vb[:], in_=vt[:])
                nc.sync.dma_start(
                    out=vb_dram[j * 128:(j + 1) * 128, :], in_=vb[:])
            # feature-major bf16 keys via DMA xbar transpose
            for k in range(c.KC):
                nc.sync.dma_start(
                    out=knT[:, k, :],
                    in_=knb_dram[:, k * 128:(k + 1) * 128], transpose=True)

        # ========= Phase 3: sim + topk + rescue + retrieve =========
        with ExitStack() as ph:
            qt_pool = ph.enter_context(tc.tile_pool(name="qt", bufs=2))
            qq_pool = ph.enter_context(tc.tile_pool(name="qq", bufs=2))
            sim_pool = ph.enter_context(tc.tile_pool(name="simt", bufs=1))
            kv_pool = ph.enter_context(tc.tile_pool(name="kv", bufs=1))
            sm_pool = ph.enter_context(tc.tile_pool(name="sm", bufs=2))
            ret_pool = ph.enter_context(tc.tile_pool(name="ret", bufs=1))
            sim_ps = ph.enter_context(
                tc.tile_pool(name="sim_ps", bufs=6, space="PSUM"))

            NCAND = c.NC
            half = NCAND // 2
            for m in range(c.MT):
                qtb = qt_pool.tile([128, c.KC, 128], BF16, tag="qtb")
                nc.sync.dma_start(
                    out=qtb[:],
                    in_=qt_b16.rearrange("(k p) n -> p k n", p=128)
                    [:, :, m * 128:(m + 1) * 128])
                qqm_t = qq_pool.tile([128, c.F], F32, tag="qqm")
                nc.sync.dma_start(
                    out=qqm_t[:], in_=qqm_dram[m * 128:(m + 1) * 128, :])

                sim_t = sim_pool.tile([128, c.D], F32, tag="sim")
                for d in range(c.DB):
                    ds = slice(d * 512, (d + 1) * 512)
                    ps = sim_ps.tile([128, 512], F32, tag="sim_ps")
                    for k in range(c.KC):
                        nc.tensor.matmul(
                            out=ps[:], lhsT=qtb[:, k, :],
                            rhs=knT[:, k, ds],
                            start=(k == 0), stop=(k == c.KC - 1))
                    nc.scalar.activation(
                        out=sim_t[:, ds], in_=ps[:],
                        func=AF.Copy, scale=rq_col[:, m:m + 1])
                    nc.sync.dma_start(
                        out=sim_out[m * 128:(m + 1) * 128, ds],
                        in_=sim_t[:, ds])

                # top-8 (descending) values + indices per row
                vals8 = sm_pool.tile([128, 8], F32, tag="vals8")
                nc.vector.max(out=vals8[:], in_=sim_t[:])
                idx8 = sm_pool.tile([128, 8], U32, tag="idx8")
                nc.vector.max_index(out=idx8[:], in_max=vals8[:],
                                    in_values=sim_t[:])

                # rescue: gather top-8 normalized key rows, fp32 dots
                #   product on DVE (one broadcast multiply per half),
                #   per-candidate sums via ACT accumulate
                rdots = sm_pool.tile([128, 8], F32, tag="rdots")
                for hf in range(2):
                    kvb = kv_pool.tile([128, half, c.F], F32, tag="kvb")
                    for t in range(half):
                        tt = hf * half + t
                        nc.gpsimd.indirect_dma_start(
                            out=kvb[:, t, :], out_offset=None,
                            in_=kn_dict,
                            in_offset=IndirectOffsetOnAxis(
                                ap=idx8[:, tt:tt + 1], axis=0))
                    qap = qqm_t[:]
                    q_b = bass.AP(qap.tensor, qap.offset,
                                  [qap.ap[0], [0, half], qap.ap[1]])
                    nc.vector.tensor_tensor(out=kvb[:], in0=kvb[:], in1=q_b,
                                            op=ALU.mult)
                    for t in range(half):
                        tt = hf * half + t
                        nc.scalar.activation(
                            out=kvb[:, t, :], in_=kvb[:, t, :], func=AF.Copy,
                            accum_out=rdots[:, tt:tt + 1])

                # top-5-of-8 selection + softmax weights
                sort8 = sm_pool.tile([128, 8], F32, tag="sort8")
                nc.vector.max(out=sort8[:], in_=rdots[:])
                ebias = sm_pool.tile([128, 1], F32, tag="ebias")
                nc.vector.tensor_scalar_mul(
                    ebias[:], sort8[:, 0:1], -1.0 / TEMPERATURE)
                mask = sm_pool.tile([128, NCAND], F32, tag="mask")
                nc.vector.tensor_scalar(
                    out=mask[:], in0=rdots[:, 0:NCAND],
                    scalar1=sort8[:, c.TOPK - 1:c.TOPK], scalar2=None,
                    op0=ALU.is_ge)
                ew = sm_pool.tile([128, NCAND], F32, tag="ew")
                nc.scalar.activation(
                    out=ew[:], in_=rdots[:, 0:NCAND], func=AF.Exp,
                    bias=ebias[:, 0:1], scale=1.0 / TEMPERATURE)
                ewm = sm_pool.tile([128, NCAND], F32, tag="ewm")
                nc.vector.tensor_tensor(out=ewm[:], in0=ew[:], in1=mask[:],
                                        op=ALU.mult)
                den = sm_pool.tile([128, 1], F32, tag="den")
                nc.vector.tensor_reduce(out=den[:], in_=ewm[:], axis=AX.X,
                                        op=ALU.add)
                rden = sm_pool.tile([128, 1], F32, tag="rden")
                nc.vector.reciprocal(out=rden[:], in_=den[:])
                wgt = sm_pool.tile([128, NCAND], F32, tag="wgt")
                nc.vector.tensor_scalar_mul(wgt[:], ewm[:], rden[:, 0:1])

                # gather bf16 value rows, scale by weights on ACT,
                # pairwise add-tree on DVE (bf16 in, fp32 out)
                ret_t = ret_pool.tile([128, c.V], F32, tag="ret_t")
                radd = ret_pool.tile([128, c.V], F32, tag="radd")
                s2 = ret_pool.tile([128, 2, c.V], F32, tag="s2")
                for hf in range(2):
                    kvb = kv_pool.tile([128, half, c.V], BF16, tag="kvb")
                    for t in range(half):
                        tt = hf * half + t
                        nc.gpsimd.indirect_dma_start(
                            out=kvb[:, t, :], out_offset=None,
                            in_=vb_dram,
                            in_offset=IndirectOffsetOnAxis(
                                ap=idx8[:, tt:tt + 1], axis=0))
                    for t in range(half):
                        tt = hf * half + t
                        nc.scalar.activation(
                            out=kvb[:, t, :], in_=kvb[:, t, :], func=AF.Copy,
                            scale=wgt[:, tt:tt + 1])
                    # half-sum: (V0+V2, V1+V3) then fold
                    nc.vector.tensor_tensor(
                        out=s2[:], in0=kvb[:, 0:2, :], in1=kvb[:, 2:4, :],
                        op=ALU.add)
                    dst = ret_t if hf == 0 else radd
                    nc.vector.tensor_tensor(
                        out=dst[:], in0=s2[:, 0, :], in1=s2[:, 1, :],
                        op=ALU.add)
                nc.vector.tensor_tensor(out=ret_t[:], in0=ret_t[:],
                                        in1=radd[:], op=ALU.add)
                nc.sync.dma_start(
                    out=retr_out[m * 128:(m + 1) * 128, :], in_=ret_t[:])


# ======================= host-side entry =======================

_CACHED = {}


def make_in_maps(cfg, q_feats, W1, b1, W2, b2, keys_p, values_p, n_cores=8):
    shard = q_feats.shape[0] // n_cores
    b1_t = np.ascontiguousarray(
        np.asarray(b1, np.float32).reshape(cfg.KC, 128).T)
    b2_t = np.ascontiguousarray(
        np.asarray(b2, np.float32).reshape(cfg.KC, 128).T)
    ident = np.eye(128, dtype=np.float32)
    w1 = np.ascontiguousarray(W1, np.float32)
    w2 = np.ascontiguousarray(W2, np.float32)
    keys = np.ascontiguousarray(keys_p, np.float32)
    vals = np.ascontiguousarray(values_p, np.float32)
    maps = []
    for i in range(n_cores):
        xs = np.asarray(q_feats[i * shard:(i + 1) * shard], np.float32)
        maps.append({
            "xT": np.ascontiguousarray(xs.T),
            "w1": w1, "w2": w2, "b1": b1_t, "b2": b2_t,
            "keys": keys, "values": vals, "ident": ident,
        })
    return maps


def kernel(q_feats, W1, b1, W2, b2, keys_p, values_p, topk):
    from concourse.bass_utils import run_bass_kernel_spmd

    assert int(topk) == 5, f"kernel hardcodes topk=5, got {topk}"
    N_CORES = 8
    N, F = q_feats.shape
    D = keys_p.shape[0]
    V = values_p.shape[1]
    assert (N, F, D, V) == (16384, 1024, 8192, 1024)

    cfg = Cfg()
    if "nc" not in _CACHED:
        _CACHED["nc"] = build(cfg)
    nc = _CACHED["nc"]

    in_maps = make_in_maps(cfg, q_feats, W1, b1, W2, b2, keys_p, values_p,
                           N_CORES)
    res = run_bass_kernel_spmd(nc, in_maps, core_ids=list(range(N_CORES)))
    sim = np.concatenate(
        [res.results[i]["sim_out"] for i in range(N_CORES)], axis=0)
    retr = np.concatenate(
        [res.results[i]["retr_out"] for i in range(N_CORES)], axis=0)
    return retr.astype(np.float32, copy=False), sim.astype(np.float32,
                                                           copy=False)


# revision 24
# speedup vs baseline: 1.0463x; 1.0173x over previous
"""Trainium2 Bass kernel for DictionaryModule (retrieval_knn).

Reference computation (per query row x of q_feats):
    h      = relu(x @ W1 + b1)
    qp     = h @ W2 + b2
    q      = qp / ||qp||
    k      = keys / ||keys|| (row-wise)
    sim    = q @ k.T                      [N, D]   (output 2)
    top5   = top_k(sim, 5)
    w      = softmax(top5_vals / 0.15)
    retr   = sum_t w_t * values[top5_idx_t]        (output 1)

Sharding: pure data-parallel over 8 NeuronCores; queries split N/8 per core,
dictionary + MLP weights replicated on every core.  No collectives.

Per-core plan:
  - MLP in fp32 on the PE with activations feature-major (x^T streamed in
    pre-transposed from the host), so W1/W2 native [in, out] are directly
    the stationary lhsT and no activation transposes are needed.
  - Query norms via square + ones-vector matmul (partition reduction on PE),
    applied as per-partition 1/||q|| scales at PSUM eviction.
  - Keys normalized dict-major (ACT square+accum), stored fp32 to DRAM for
    the rescue pass, and PE-transposed into a resident feature-major bf16
    copy for the sim matmul.
  - sim = q^T.T @ knT in bf16 (fp32 accumulate).  bf16 sim error (~2e-4) is
    >> fp32 top-k decision gaps, so the top-6 bf16 candidates (via the DVE
    max/max_index top-8 instructions) are re-scored exactly in fp32
    (indirect-DMA gather of normalized key rows + DVE dots), then top-5-of-6
    selected by threshold, softmaxed, and applied to gathered value rows.
"""

import sys

sys.path.insert(0, "/opt/trn_rl_repo")

from contextlib import ExitStack

import numpy as np

import concourse.bass as bass
import concourse.tile as tile
from concourse import bacc, mybir
from concourse.bass import IndirectOffsetOnAxis

F32 = mybir.dt.float32
BF16 = mybir.dt.bfloat16
U32 = mybir.dt.uint32
AF = mybir.ActivationFunctionType
ALU = mybir.AluOpType
AX = mybir.AxisListType

TEMPERATURE = 0.15
NEG_BIG = -1.0e30


class Cfg:
    def __init__(self, nq=2048, feat=1024, dict_size=8192, val=1024, topk=5,
                 n_cand=8):
        assert nq % 256 == 0 and feat % 128 == 0
        assert dict_size % 512 == 0 and val % 128 == 0
        self.NQ = nq              # queries per core
        self.F = feat             # feature/key dim
        self.D = dict_size        # dictionary entries
        self.V = val              # value dim
        self.TOPK = topk
        self.NC = n_cand          # rescued candidates (topk < n_cand <= 8)
        assert topk < n_cand <= 8 and n_cand % 2 == 0
        self.MT = nq // 128       # query tiles
        self.KC = feat // 128     # feature chunks (contraction)
        self.DB = dict_size // 512    # dict blocks of 512
        self.DT = dict_size // 128    # dict tiles of 128 rows
        self.HQ = nq // 2         # MLP query half-width
        self.SL = min(512, self.HQ)   # MLP moving-dim slice
        assert self.HQ % self.SL == 0
        self.NSH = self.HQ // self.SL  # slices per half
        self.MTH = self.HQ // 128      # query tiles per half
        self.QW = min(2048, dict_size)  # top-k scan quarter width
        assert dict_size % self.QW == 0 and self.QW % 512 == 0
        self.NQR = dict_size // self.QW  # scan quarters
        self.NCAT = self.NQR * 8         # merge candidates


def build(cfg: Cfg):
    """Builds the single-core SPMD program. Returns the bacc module."""
    nc = bacc.Bacc("TRN2", target_bir_lowering=False, debug=False)
    c = cfg

    io = {}
    io["xT"] = nc.dram_tensor("xT", [c.F, c.NQ], F32, kind="ExternalInput").ap()
    io["w1"] = nc.dram_tensor("w1", [c.F, c.F], F32, kind="ExternalInput").ap()
    io["w2"] = nc.dram_tensor("w2", [c.F, c.F], F32, kind="ExternalInput").ap()
    io["b1"] = nc.dram_tensor("b1", [128, c.KC], F32, kind="ExternalInput").ap()
    io["b2"] = nc.dram_tensor("b2", [128, c.KC], F32, kind="ExternalInput").ap()
    io["keys"] = nc.dram_tensor("keys", [c.D, c.F], F32,
                                kind="ExternalInput").ap()
    io["values"] = nc.dram_tensor("values", [c.D, c.V], F32,
                                  kind="ExternalInput").ap()
    io["ident"] = nc.dram_tensor("ident", [128, 128], F32,
                                 kind="ExternalInput").ap()
    io["pidx32"] = nc.dram_tensor("pidx32", [128, 1], U32,
                                  kind="ExternalInput").ap()
    io["qoff"] = nc.dram_tensor("qoff", [128, c.NCAT], U32,
                                kind="ExternalInput").ap()
    io["sim_out"] = nc.dram_tensor("sim_out", [c.NQ, c.D], F32,
                                   kind="ExternalOutput").ap()
    io["retr_out"] = nc.dram_tensor("retr_out", [c.NQ, c.V], F32,
                                    kind="ExternalOutput").ap()
    # DRAM scratch
    io["kn_dict"] = nc.dram_tensor("kn_dict", [c.D, c.F], F32).ap()
    io["knb_dram"] = nc.dram_tensor("knb_dram", [c.D, c.F], BF16).ap()
    io["vb_dram"] = nc.dram_tensor("vb_dram", [c.D, c.V], BF16).ap()
    io["qt_b16"] = nc.dram_tensor("qt_b16", [c.F, c.NQ], BF16).ap()
    io["qqm_dram"] = nc.dram_tensor("qqm_dram", [c.NQ, c.F], F32).ap()
    io["ssq_dram"] = nc.dram_tensor("ssq_dram", [c.MT, 128], F32).ap()
    io["idx_dram"] = [nc.dram_tensor(f"idx_dram{i}", [128 * c.NCAT, 1],
                                     U32).ap()
                      for i in range(2)]

    with tile.TileContext(nc) as tc:
        _emit(tc, c, io)
    nc.compile()
    return nc


def _emit(tc, c, io):
    nc = tc.nc
    xT, w1, w2, b1, b2 = io["xT"], io["w1"], io["w2"], io["b1"], io["b2"]
    keys, values, ident = io["keys"], io["values"], io["ident"]
    sim_out, retr_out = io["sim_out"], io["retr_out"]
    kn_dict, qt_b16 = io["kn_dict"], io["qt_b16"]
    qqm_dram, ssq_dram = io["qqm_dram"], io["ssq_dram"]
    knb_dram, vb_dram = io["knb_dram"], io["vb_dram"]
    pidx32, qoff, idx_dram = io["pidx32"], io["qoff"], io["idx_dram"]

    with ExitStack() as top:
        const_pool = top.enter_context(tc.tile_pool(name="const", bufs=1))
        rq_pool = top.enter_context(tc.tile_pool(name="rq", bufs=1))

        ident_t = const_pool.tile([128, 128], F32, tag="ident")
        nc.sync.dma_start(out=ident_t[:], in_=ident[:, :])
        ones_t = const_pool.tile([128, 1], F32, tag="ones")
        nc.vector.memset(ones_t[:], 1.0)

        # per-query 1/||q||, column layout: rq_col[p, m] = 1/||q_{m*128+p}||
        rq_col = rq_pool.tile([128, c.MT], F32, tag="rq_col")

        # ================= Phase 1: MLP (fp32, feature-major) =============
        with ExitStack() as ph:
            x_pool = ph.enter_context(tc.tile_pool(name="x", bufs=1))
            h_pool = ph.enter_context(tc.tile_pool(name="h", bufs=1))
            qp_pool = ph.enter_context(tc.tile_pool(name="qp", bufs=1))
            wgt_pool = ph.enter_context(tc.tile_pool(name="wgt", bufs=2))
            bias_pool = ph.enter_context(tc.tile_pool(name="bias", bufs=1))
            sq_pool = ph.enter_context(tc.tile_pool(name="sq", bufs=1))
            qtb_pool = ph.enter_context(tc.tile_pool(name="qtb", bufs=2))
            ssq_pool = ph.enter_context(tc.tile_pool(name="ssq", bufs=1))
            qqm_pool = ph.enter_context(tc.tile_pool(name="qqm", bufs=2))
            mlp_ps = ph.enter_context(
                tc.tile_pool(name="mlp_ps", bufs=2, space="PSUM"))
            ssq_ps = ph.enter_context(
                tc.tile_pool(name="ssq_ps", bufs=1, space="PSUM"))
            tr_ps = ph.enter_context(
                tc.tile_pool(name="tr_ps", bufs=2, space="PSUM"))

            b1_sb = bias_pool.tile([128, c.KC], F32, tag="b1")
            nc.sync.dma_start(out=b1_sb[:], in_=b1[:, :])
            b2_sb = bias_pool.tile([128, c.KC], F32, tag="b2")
            nc.sync.dma_start(out=b2_sb[:], in_=b2[:, :])

            for hq in range(2):
                qcols = slice(hq * c.HQ, (hq + 1) * c.HQ)
                x_sb = x_pool.tile([128, c.KC, c.HQ], F32, tag="x")
                nc.sync.dma_start(
                    out=x_sb[:],
                    in_=xT.rearrange("(k p) n -> p k n", p=128)[:, :, qcols])

                # ---- layer 1 ----
                h_sb = h_pool.tile([128, c.KC, c.HQ], F32, tag="h")
                for o in range(c.KC):
                    wt = wgt_pool.tile([128, c.KC, 128], F32, tag="w")
                    nc.sync.dma_start(
                        out=wt[:],
                        in_=w1.rearrange("(k p) o -> p k o", p=128)
                        [:, :, o * 128:(o + 1) * 128])
                    for n in range(c.NSH):
                        ns = slice(n * c.SL, (n + 1) * c.SL)
                        ps = mlp_ps.tile([128, c.SL], F32, tag="mlp_ps")
                        for k in range(c.KC):
                            nc.tensor.matmul(
                                out=ps[:], lhsT=wt[:, k, :],
                                rhs=x_sb[:, k, ns],
                                start=(k == 0), stop=(k == c.KC - 1))
                        nc.scalar.activation(
                            out=h_sb[:, o, ns], in_=ps[:],
                            func=AF.Relu, bias=b1_sb[:, o:o + 1], scale=1.0)

                # ---- layer 2 (+ query squared-norms via ones-matmul) ----
                qp_sb = qp_pool.tile([128, c.KC, c.HQ], F32, tag="qp")
                sq_sb = sq_pool.tile([128, c.HQ], F32, tag="sq")
                ssq_psum = [ssq_ps.tile([1, c.SL], F32, tag=f"ssq{n}",
                                        name=f"ssq_psum{n}")
                            for n in range(c.NSH)]
                for o in range(c.KC):
                    wt = wgt_pool.tile([128, c.KC, 128], F32, tag="w")
                    nc.sync.dma_start(
                        out=wt[:],
                        in_=w2.rearrange("(k p) o -> p k o", p=128)
                        [:, :, o * 128:(o + 1) * 128])
                    for n in range(c.NSH):
                        ns = slice(n * c.SL, (n + 1) * c.SL)
                        ps = mlp_ps.tile([128, c.SL], F32, tag="mlp_ps")
                        for k in range(c.KC):
                            nc.tensor.matmul(
                                out=ps[:], lhsT=wt[:, k, :],
                                rhs=h_sb[:, k, ns],
                                start=(k == 0), stop=(k == c.KC - 1))
                        nc.scalar.activation(
                            out=qp_sb[:, o, ns], in_=ps[:],
                            func=AF.Identity, bias=b2_sb[:, o:o + 1],
                            scale=1.0)
                    nc.vector.tensor_tensor(
                        out=sq_sb[:], in0=qp_sb[:, o, :], in1=qp_sb[:, o, :],
                        op=ALU.mult)
                    for n in range(c.NSH):
                        ns = slice(n * c.SL, (n + 1) * c.SL)
                        nc.tensor.matmul(
                            out=ssq_psum[n][:], lhsT=ones_t[:],
                            rhs=sq_sb[:, ns],
                            start=(o == 0), stop=(o == c.KC - 1))

                # ssq -> rq_col slice for this half (via DRAM re-layout)
                ssq_row = ssq_pool.tile([1, c.HQ], F32, tag="ssq_row")
                for n in range(c.NSH):
                    nc.scalar.activation(
                        out=ssq_row[:, n * c.SL:(n + 1) * c.SL],
                        in_=ssq_psum[n][:], func=AF.Copy)
                nc.sync.dma_start(
                    out=ssq_dram[hq * c.MTH:(hq + 1) * c.MTH, :],
                    in_=ssq_row[:])
                ssq_col = ssq_pool.tile([128, c.MTH], F32, tag="ssq_col")
                nc.sync.dma_start(
                    out=ssq_col[:],
                    in_=ssq_dram[hq * c.MTH:(hq + 1) * c.MTH, :]
                    .rearrange("m p -> p m"))
                nrm_col = ssq_pool.tile([128, c.MTH], F32, tag="nrm_col")
                nc.scalar.activation(out=nrm_col[:], in_=ssq_col[:],
                                     func=AF.Sqrt)
                nc.vector.reciprocal(
                    out=rq_col[:, hq * c.MTH:(hq + 1) * c.MTH],
                    in_=nrm_col[:])

                # q^T -> bf16 -> DRAM (sim lhsT, streamed back per m-block)
                for k in range(c.KC):
                    qtb_sb = qtb_pool.tile([128, c.HQ], BF16, tag="qtb")
                    nc.vector.tensor_copy(out=qtb_sb[:], in_=qp_sb[:, k, :])
                    nc.sync.dma_start(
                        out=qt_b16[k * 128:(k + 1) * 128, qcols],
                        in_=qtb_sb[:])

                # query-major normalized q -> DRAM (fp32 rescue operand)
                for mh in range(c.MTH):
                    m = hq * c.MTH + mh
                    qqm_t = qqm_pool.tile([128, c.F], F32, tag="qqm")
                    for f in range(c.KC):
                        pt = tr_ps.tile([128, 128], F32, tag="tr")
                        nc.tensor.transpose(
                            out=pt[:],
                            in_=qp_sb[:, f, mh * 128:(mh + 1) * 128],
                            identity=ident_t[:])
                        nc.scalar.activation(
                            out=qqm_t[:, f * 128:(f + 1) * 128], in_=pt[:],
                            func=AF.Copy, scale=rq_col[:, m:m + 1])
                    nc.sync.dma_start(
                        out=qqm_dram[m * 128:(m + 1) * 128, :], in_=qqm_t[:])

        # ===================== Phases 2+3 (knT resident) =====================
        knT_pool = top.enter_context(tc.tile_pool(name="knT", bufs=1))
        # resident feature-major normalized keys (bf16): [128, KC, D]
        knT = knT_pool.tile([128, c.KC, c.D], BF16, tag="knT")

        # ======= Phase 2: key normalize + bf16 tables (no PE work) =======
        with ExitStack() as ph:
            kt_pool = ph.enter_context(tc.tile_pool(name="kt", bufs=3))
            kn_pool = ph.enter_context(tc.tile_pool(name="kn", bufs=3))
            ksc_pool = ph.enter_context(tc.tile_pool(name="ksc", bufs=3))
            vt_pool = ph.enter_context(tc.tile_pool(name="vt", bufs=3))
            for j in range(c.DT):
                kt = kt_pool.tile([128, c.F], F32, tag="kt")
                nc.sync.dma_start(
                    out=kt[:], in_=keys[j * 128:(j + 1) * 128, :])
                ksq = ksc_pool.tile([128, c.F], F32, tag="ksq")
                kssq = ksc_pool.tile([128, 1], F32, tag="kssq")
                nc.scalar.activation(out=ksq[:], in_=kt[:], func=AF.Square,
                                     accum_out=kssq[:])
                knorm = ksc_pool.tile([128, 1], F32, tag="knorm")
                nc.scalar.activation(out=knorm[:], in_=kssq[:], func=AF.Sqrt)
                krk = ksc_pool.tile([128, 1], F32, tag="krk")
                nc.vector.reciprocal(out=krk[:], in_=knorm[:])
                kn = kn_pool.tile([128, c.F], F32, tag="kn")
                nc.scalar.activation(out=kn[:], in_=kt[:], func=AF.Copy,
                                     scale=krk[:, 0:1])
                nc.sync.dma_start(
                    out=kn_dict[j * 128:(j + 1) * 128, :], in_=kn[:])
                knb = kn_pool.tile([128, c.F], BF16, tag="knb")
                nc.vector.tensor_copy(out=knb[:], in_=kn[:])
                nc.sync.dma_start(
                    out=knb_dram[j * 128:(j + 1) * 128, :], in_=knb[:])
                vt = vt_pool.tile([128, c.V], F32, tag="vt")
                nc.sync.dma_start(
                    out=vt[:], in_=values[j * 128:(j + 1) * 128, :])
                vb = vt_pool.tile([128, c.V], BF16, tag="vb")
                nc.vector.tensor_copy(out=vb[:], in_=vt[:])
                nc.sync.dma_start(
                    out=vb_dram[j * 128:(j + 1) * 128, :], in_=vb[:])
            # feature-major bf16 keys via DMA xbar transpose
            for k in range(c.KC):
                nc.sync.dma_start(
                    out=knT[:, k, :],
                    in_=knb_dram[:, k * 128:(k + 1) * 128], transpose=True)

        # ========= Phase 3: sim + topk + rescue + retrieve =========
        with ExitStack() as ph:
            qt_pool = ph.enter_context(tc.tile_pool(name="qt", bufs=2))
            qq_pool = ph.enter_context(tc.tile_pool(name="qq", bufs=2))
            sq_pool = ph.enter_context(tc.tile_pool(name="sq3", bufs=3))
            kv_pool = ph.enter_context(tc.tile_pool(name="kv", bufs=1))
            sm_pool = ph.enter_context(tc.tile_pool(name="sm", bufs=2))
            cat_pool = ph.enter_context(tc.tile_pool(name="cat", bufs=2))
            ret_pool = ph.enter_context(tc.tile_pool(name="ret", bufs=1))
            cst_pool = ph.enter_context(tc.tile_pool(name="cst3", bufs=1))
            sim_ps = ph.enter_context(
                tc.tile_pool(name="sim_ps", bufs=6, space="PSUM"))

            pidx_t = cst_pool.tile([128, 1], U32, tag="pidx")
            nc.sync.dma_start(out=pidx_t[:], in_=pidx32[:, :])
            qoff_t = cst_pool.tile([128, c.NCAT], U32, tag="qoff")
            nc.sync.dma_start(out=qoff_t[:], in_=qoff[:, :])

            NCAND = c.NC
            half = NCAND // 2
            DPQ = c.QW // 512  # d-blocks per scan quarter
            for m in range(c.MT):
                qtb = qt_pool.tile([128, c.KC, 128], BF16, tag="qtb")
                nc.sync.dma_start(
                    out=qtb[:],
                    in_=qt_b16.rearrange("(k p) n -> p k n", p=128)
                    [:, :, m * 128:(m + 1) * 128])
                qqm_t = qq_pool.tile([128, c.F], F32, tag="qqm")
                nc.sync.dma_start(
                    out=qqm_t[:], in_=qqm_dram[m * 128:(m + 1) * 128, :])

                # sim matmuls in scan-quarters; per-quarter top-8 scans
                vals_cat = cat_pool.tile([128, c.NCAT], F32, tag="vals_cat")
                idx_cat = cat_pool.tile([128, c.NCAT], U32, tag="idx_cat")
                for q in range(c.NQR):
                    sq_t = sq_pool.tile([128, c.QW], F32, tag="sq3")
                    for dq in range(DPQ):
                        d = q * DPQ + dq
                        ps = sim_ps.tile([128, 512], F32, tag="sim_ps")
                        for k in range(c.KC):
                            nc.tensor.matmul(
                                out=ps[:], lhsT=qtb[:, k, :],
                                rhs=knT[:, k, d * 512:(d + 1) * 512],
                                start=(k == 0), stop=(k == c.KC - 1))
                        nc.scalar.activation(
                            out=sq_t[:, dq * 512:(dq + 1) * 512], in_=ps[:],
                            func=AF.Copy, scale=rq_col[:, m:m + 1])
                    nc.sync.dma_start(
                        out=sim_out[m * 128:(m + 1) * 128,
                                    q * c.QW:(q + 1) * c.QW],
                        in_=sq_t[:])
                    nc.vector.max(out=vals_cat[:, q * 8:(q + 1) * 8],
                                  in_=sq_t[:])
                    nc.vector.max_index(
                        out=idx_cat[:, q * 8:(q + 1) * 8],
                        in_max=vals_cat[:, q * 8:(q + 1) * 8],
                        in_values=sq_t[:])
                # global quarter offsets, then merge the NQR top-8 lists
                nc.vector.tensor_tensor(out=idx_cat[:], in0=idx_cat[:],
                                        in1=qoff_t[:], op=ALU.add)
                vals8 = sm_pool.tile([128, 8], F32, tag="vals8")
                nc.vector.max(out=vals8[:], in_=vals_cat[:])
                pos8 = sm_pool.tile([128, 8], U32, tag="pos8")
                nc.vector.max_index(out=pos8[:], in_max=vals8[:],
                                    in_values=vals_cat[:])
                # resolve winner indices via DRAM bounce + 1-elem gathers
                idxd = idx_dram[m % 2]
                nc.sync.dma_start(
                    out=idxd.rearrange("(p j) o -> p (j o)", p=128),
                    in_=idx_cat[:])
                off8 = sm_pool.tile([128, 8], U32, tag="off8")
                pap = pidx_t[:]
                pidx_b = bass.AP(pap.tensor, pap.offset, [pap.ap[0], [0, 8]])
                nc.vector.tensor_tensor(out=off8[:], in0=pos8[:],
                                        in1=pidx_b, op=ALU.add)
                idx8 = sm_pool.tile([128, 8], U32, tag="idx8")
                for t in range(8):
                    nc.gpsimd.indirect_dma_start(
                        out=idx8[:, t:t + 1], out_offset=None,
                        in_=idxd,
                        in_offset=IndirectOffsetOnAxis(
                            ap=off8[:, t:t + 1], axis=0))

                # rescue: gather top-8 normalized key rows, fp32 dots
                #   product on DVE (one broadcast multiply per half),
                #   per-candidate sums via ACT accumulate
                rdots = sm_pool.tile([128, 8], F32, tag="rdots")
                for hf in range(2):
                    kvb = kv_pool.tile([128, half, c.F], F32, tag="kvb")
                    for t in range(half):
                        tt = hf * half + t
                        nc.gpsimd.indirect_dma_start(
                            out=kvb[:, t, :], out_offset=None,
                            in_=kn_dict,
                            in_offset=IndirectOffsetOnAxis(
                                ap=idx8[:, tt:tt + 1], axis=0))
                    qap = qqm_t[:]
                    q_b = bass.AP(qap.tensor, qap.offset,
                                  [qap.ap[0], [0, half], qap.ap[1]])
                    nc.vector.tensor_tensor(out=kvb[:], in0=kvb[:], in1=q_b,
                                            op=ALU.mult)
                    for t in range(half):
                        tt = hf * half + t
                        nc.scalar.activation(
                            out=kvb[:, t, :], in_=kvb[:, t, :], func=AF.Copy,
                            accum_out=rdots[:, tt:tt + 1])

                # top-5-of-8 selection + softmax weights
                sort8 = sm_pool.tile([128, 8], F32, tag="sort8")
                nc.vector.max(out=sort8[:], in_=rdots[:])
                ebias = sm_pool.tile([128, 1], F32, tag="ebias")
                nc.vector.tensor_scalar_mul(
                    ebias[:], sort8[:, 0:1], -1.0 / TEMPERATURE)
                mask = sm_pool.tile([128, NCAND], F32, tag="mask")
                nc.vector.tensor_scalar(
                    out=mask[:], in0=rdots[:, 0:NCAND],
                    scalar1=sort8[:, c.TOPK - 1:c.TOPK], scalar2=None,
                    op0=ALU.is_ge)
                ew = sm_pool.tile([128, NCAND], F32, tag="ew")
                nc.scalar.activation(
                    out=ew[:], in_=rdots[:, 0:NCAND], func=AF.Exp,
                    bias=ebias[:, 0:1], scale=1.0 / TEMPERATURE)
                ewm = sm_pool.tile([128, NCAND], F32, tag="ewm")
                nc.vector.tensor_tensor(out=ewm[:], in0=ew[:], in1=mask[:],
                                        op=ALU.mult)
                den = sm_pool.tile([128, 1], F32, tag="den")
                nc.vector.tensor_reduce(out=den[:], in_=ewm[:], axis=AX.X,
                                        op=ALU.add)
                rden = sm_pool.tile([128, 1], F32, tag="rden")
                nc.vector.reciprocal(out=rden[:], in_=den[:])
                wgt = sm_pool.tile([128, NCAND], F32, tag="wgt")
                nc.vector.tensor_scalar_mul(wgt[:], ewm[:], rden[:, 0:1])

                # gather bf16 value rows, scale by weights on ACT,
                # pairwise add-tree on DVE (bf16 in, fp32 out)
                ret_t = ret_pool.tile([128, c.V], F32, tag="ret_t")
                radd = ret_pool.tile([128, c.V], F32, tag="radd")
                s2 = ret_pool.tile([128, 2, c.V], F32, tag="s2")
                for hf in range(2):
                    kvb = kv_pool.tile([128, half, c.V], BF16, tag="kvb")
                    for t in range(half):
                        tt = hf * half + t
                        nc.gpsimd.indirect_dma_start(
                            out=kvb[:, t, :], out_offset=None,
                            in_=vb_dram,
                            in_offset=IndirectOffsetOnAxis(
                                ap=idx8[:, tt:tt + 1], axis=0))
                    for t in range(half):
                        tt = hf * half + t
                        nc.scalar.activation(
                            out=kvb[:, t, :], in_=kvb[:, t, :], func=AF.Copy,
                            scale=wgt[:, tt:tt + 1])
                    # half-sum: (V0+V2, V1+V3) then fold
                    nc.vector.tensor_tensor(
                        out=s2[:], in0=kvb[:, 0:2, :], in1=kvb[:, 2:4, :],
                        op=ALU.add)
                    dst = ret_t if hf == 0 else radd
                    nc.vector.tensor_tensor(
                        out=dst[:], in0=s2[:, 0, :], in1=s2[:, 1, :],
                        op=ALU.add)
                nc.vector.tensor_tensor(out=ret_t[:], in0=ret_t[:],
                                        in1=radd[:], op=ALU.add)
                nc.sync.dma_start(
                    out=retr_out[m * 128:(m + 1) * 128, :], in_=ret_t[:])


# ======================= host-side entry =======================

_CACHED = {}


def make_in_maps(cfg, q_feats, W1, b1, W2, b2, keys_p, values_p, n_cores=8):
    shard = q_feats.shape[0] // n_cores
    b1_t = np.ascontiguousarray(
        np.asarray(b1, np.float32).reshape(cfg.KC, 128).T)
    b2_t = np.ascontiguousarray(
        np.asarray(b2, np.float32).reshape(cfg.KC, 128).T)
    ident = np.eye(128, dtype=np.float32)
    pidx32 = (np.arange(128, dtype=np.uint32) * cfg.NCAT).reshape(128, 1)
    qoff = np.broadcast_to(
        (np.arange(cfg.NCAT, dtype=np.uint32) // 8) * cfg.QW,
        (128, cfg.NCAT)).copy()
    w1 = np.ascontiguousarray(W1, np.float32)
    w2 = np.ascontiguousarray(W2, np.float32)
    keys = np.ascontiguousarray(keys_p, np.float32)
    vals = np.ascontiguousarray(values_p, np.float32)
    maps = []
    for i in range(n_cores):
        xs = np.asarray(q_feats[i * shard:(i + 1) * shard], np.float32)
        maps.append({
            "xT": np.ascontiguousarray(xs.T),
            "w1": w1, "w2": w2, "b1": b1_t, "b2": b2_t,
            "keys": keys, "values": vals, "ident": ident,
            "pidx32": pidx32, "qoff": qoff,
        })
    return maps


def kernel(q_feats, W1, b1, W2, b2, keys_p, values_p, topk):
    from concourse.bass_utils import run_bass_kernel_spmd

    assert int(topk) == 5, f"kernel hardcodes topk=5, got {topk}"
    N_CORES = 8
    N, F = q_feats.shape
    D = keys_p.shape[0]
    V = values_p.shape[1]
    assert (N, F, D, V) == (16384, 1024, 8192, 1024)

    cfg = Cfg()
    if "nc" not in _CACHED:
        _CACHED["nc"] = build(cfg)
    nc = _CACHED["nc"]

    in_maps = make_in_maps(cfg, q_feats, W1, b1, W2, b2, keys_p, values_p,
                           N_CORES)
    res = run_bass_kernel_spmd(nc, in_maps, core_ids=list(range(N_CORES)))
    sim = np.concatenate(
        [res.results[i]["sim_out"] for i in range(N_CORES)], axis=0)
    retr = np.concatenate(
        [res.results[i]["retr_out"] for i in range(N_CORES)], axis=0)
    return retr.astype(np.float32, copy=False), sim.astype(np.float32,
                                                           copy=False)
